# revision 1
# baseline (speedup 1.0000x reference)
"""Trainium2 Bass kernel for retrieval-knn attention classifier (nn_MA_51866025067137).

Strategy (8 NeuronCores):
  Phase 1 — memory_keys sharded along N (12800 keys/core, padded 100000->102400
  with dummy rows).  Each core computes cosine-similarity ranking values for all
  256 queries against its shard (fp32r matmuls on the PE; keys pre-normalized so
  the matmul directly yields cosine ranking values) and extracts its local
  top-32 (value, index) per query with DVE max8/max_index/match_replace, using a
  low-9-mantissa-bit packing trick to recover in-segment indices.
  Host — merges the 8x40 candidates per row, re-scores them exactly in fp32,
  and gathers the global top-32 key vectors.
  Phase 2 — batch sharded (32 queries/core): memory-attention module
  (tanh(qWq + knnWm + b) -> scores -> softmax -> weighted sum) and classifier,
  via small fp32r matmuls; the softmax-weighted sum is a block-diagonal matmul.
"""

import numpy as np

import concourse.bacc as bacc
import concourse.mybir as mybir
from concourse.tile import TileContext
from concourse.bass_utils import run_bass_kernel_spmd
from concourse.masks import make_identity

# problem dims (hardcoded per harness contract)
B, N, D = 256, 100000, 512
A, C, K = 256, 100, 32
NC_CORES = 8
NPAD = 102400             # 8 * 12800
SHARD = NPAD // NC_CORES  # 12800
CHUNK = 512               # keys per inner loop step
NCHUNK = SHARD // CHUNK   # 25
SEG = 512                 # max8 segment width (9-bit in-segment index)
NSEG = SHARD // SEG       # 25
L1W = NSEG * 8            # 200
BROWS = B // NC_CORES     # 32 rows per core in phase 2
KLOC = 40                 # local candidates per core per row
CAND = NC_CORES * KLOC    # 320 merged candidates per row

f32 = mybir.dt.float32
f32r = mybir.dt.float32r
u32 = mybir.dt.uint32

_PH1 = None
_PH2 = None


def _build_phase1():
    nc = bacc.Bacc("TRN2", target_bir_lowering=False)
    khatT = nc.dram_tensor("khatT", [NCHUNK, 128, 4 * CHUNK], f32r, kind="ExternalInput")
    qT = nc.dram_tensor("qT", [D, B], f32r, kind="ExternalInput")
    win_out = nc.dram_tensor("win", [B, KLOC], f32, kind="ExternalOutput")
    pos_out = nc.dram_tensor("pos", [B, KLOC], u32, kind="ExternalOutput")

    with TileContext(nc) as tc:
        with (
            tc.tile_pool(name="const", bufs=1) as constp,
            tc.tile_pool(name="qpool", bufs=1) as qpool,
            tc.tile_pool(name="keys", bufs=6) as keyp,
            tc.tile_pool(name="packed", bufs=8) as packp,
            tc.tile_pool(name="l1", bufs=1) as l1p,
            tc.tile_pool(name="small", bufs=1) as smallp,
            tc.tile_pool(name="psum", bufs=2, space="PSUM") as psump,
        ):
            # constants: AND-mask (0xFFFFFE00) per partition; iota 0..511
            mask_t = constp.tile([128, 1], u32, tag="mask")
            nc.vector.memset(mask_t[:], 0xFFFFFE00)
            iota_t = constp.tile([128, CHUNK], u32, tag="iota")
            nc.gpsimd.iota(iota_t[:], pattern=[[1, CHUNK]], base=0,
                           channel_multiplier=0)

            # load qT and relu in place
            qTr = []
            for dc in range(4):
                t = qpool.tile([128, B], f32r, tag=f"qt{dc}")
                nc.sync.dma_start(out=t[:], in_=qT[dc * 128:(dc + 1) * 128, :])
                nc.scalar.activation(t[:], t[:], mybir.ActivationFunctionType.Relu)
                qTr.append(t)

            L1 = [l1p.tile([128, L1W], f32, tag=f"l1_{qt}", name=f"l1_{qt}") for qt in range(2)]

            for c in range(NCHUNK):
                kt = keyp.tile([128, 4 * CHUNK], f32r, tag="kt")
                nc.sync.dma_start(out=kt[:], in_=khatT[c, :, :])
                for qt in range(2):
                    ps = psump.tile([128, CHUNK], f32, tag=f"sim{qt}")
                    for dc in range(4):
                        nc.tensor.matmul(
                            ps[:],
                            lhsT=qTr[dc][:, qt * 128:(qt + 1) * 128],
                            rhs=kt[:, dc * CHUNK:(dc + 1) * CHUNK],
                            start=(dc == 0), stop=(dc == 3),
                        )
                    # evict (ACT), pack on GPSIMD: packed = (sim & mask) | iota
                    ev = packp.tile([128, CHUNK], f32, tag=f"ev{qt}")
                    nc.scalar.copy(out=ev[:], in_=ps[:])
                    pk = packp.tile([128, CHUNK], f32, tag=f"pk{qt}")
                    nc.vector.scalar_tensor_tensor(
                        out=pk[:].bitcast(u32), in0=ev[:].bitcast(u32),
                        scalar=mask_t[:], in1=iota_t[:],
                        op0=mybir.AluOpType.bitwise_and,
                        op1=mybir.AluOpType.bitwise_or,
                    )
                    nc.vector.max(out=L1[qt][:, c * 8:(c + 1) * 8], in_=pk[:])

            # extraction: 5 rounds of top-8 from L1 (400 wide)
            for qt in range(2):
                win = smallp.tile([128, KLOC], f32, tag=f"win{qt}")
                pos = smallp.tile([128, KLOC], u32, tag=f"pos{qt}")
                for r in range(5):
                    w8 = win[:, r * 8:(r + 1) * 8]
                    nc.vector.max(out=w8, in_=L1[qt][:])
                    nc.vector.max_index(out=pos[:, r * 8:(r + 1) * 8],
                                        in_max=w8, in_values=L1[qt][:])
                    if r < 4:
                        nc.vector.match_replace(out=L1[qt][:], in_to_replace=w8,
                                                in_values=L1[qt][:],
                                                imm_value=-3.0e38)
                nc.sync.dma_start(out=win_out[qt * 128:(qt + 1) * 128, :], in_=win[:])
                nc.sync.dma_start(out=pos_out[qt * 128:(qt + 1) * 128, :], in_=pos[:])
    nc.finalize()
    return nc


def _build_phase2():
    nc = bacc.Bacc("TRN2", target_bir_lowering=False)
    qT_in = nc.dram_tensor("qT", [D, BROWS], f32r, kind="ExternalInput")       # pre-relu
    knn_in = nc.dram_tensor("knn", [BROWS * K, D], f32r, kind="ExternalInput")
    knnT_in = nc.dram_tensor("knnT", [D, BROWS * K], f32r, kind="ExternalInput")
    Wq_in = nc.dram_tensor("Wq", [D, A], f32r, kind="ExternalInput")
    Wm_in = nc.dram_tensor("Wm", [D, A], f32r, kind="ExternalInput")
    Ws_in = nc.dram_tensor("Ws", [A, 1], f32r, kind="ExternalInput")
    bqm_in = nc.dram_tensor("bqm", [A, 1], f32, kind="ExternalInput")          # bq+bm
    Wc_in = nc.dram_tensor("Wc", [2 * D, C], f32r, kind="ExternalInput")
    out_d = nc.dram_tensor("out", [BROWS, C], f32, kind="ExternalOutput")      # +bc host
    escratch = nc.dram_tensor("escratch", [1, BROWS * K], f32)                 # bounce

    NCD = BROWS * K  # 1024

    with TileContext(nc) as tc:
        with (
            tc.tile_pool(name="big", bufs=1) as bigp,
            tc.tile_pool(name="small", bufs=1) as smallp,
            tc.tile_pool(name="psum", bufs=1, space="PSUM") as psump,
        ):
            # ---- load inputs (M-padded tiles to satisfy fp32r col_grp=0xf) ----
            qT = [smallp.tile([128, 128], f32r, tag=f"qT{dc}", name=f"qTt{dc}") for dc in range(4)]
            for dc in range(4):
                nc.vector.memset(qT[dc][:].bitcast(u32), 0)
                nc.sync.dma_start(out=qT[dc][:, :BROWS],
                                  in_=qT_in[dc * 128:(dc + 1) * 128, :])
                nc.scalar.activation(qT[dc][:, :BROWS], qT[dc][:, :BROWS],
                                     mybir.ActivationFunctionType.Relu)
            knnall = bigp.tile([128, 8 * D], f32r, tag="knnall")
            nc.sync.dma_start(out=knnall[:].rearrange("p (t d) -> p t d", t=8),
                              in_=knn_in[:].rearrange("(t p) d -> p t d", p=128))
            knn = [knnall[:, t * D:(t + 1) * D] for t in range(8)]
            knnTall = bigp.tile([128, 4 * NCD], f32r, tag="knnTall")
            nc.sync.dma_start(out=knnTall[:].rearrange("p (dc c) -> p dc c", dc=4),
                              in_=knnT_in[:].rearrange("(dc p) c -> p dc c", p=128))
            knnT = [knnTall[:, dc * NCD:(dc + 1) * NCD] for dc in range(4)]
            Wqall = smallp.tile([128, 4 * A], f32r, tag="Wqall")
            nc.sync.dma_start(out=Wqall[:].rearrange("p (dc a) -> p dc a", dc=4),
                              in_=Wq_in[:].rearrange("(dc p) a -> p dc a", p=128))
            Wq = [Wqall[:, dc * A:(dc + 1) * A] for dc in range(4)]
            Wmall = smallp.tile([128, 4 * A], f32r, tag="Wmall")
            nc.sync.dma_start(out=Wmall[:].rearrange("p (dc a) -> p dc a", dc=4),
                              in_=Wm_in[:].rearrange("(dc p) a -> p dc a", p=128))
            Wm = [Wmall[:, dc * A:(dc + 1) * A] for dc in range(4)]
            Ws = [smallp.tile([128, 128], f32r, tag=f"Ws{at}", name=f"Wst{at}") for at in range(2)]
            bqm = [smallp.tile([128, 1], f32, tag=f"bqm{at}", name=f"bqmt{at}") for at in range(2)]
            for at in range(2):
                nc.vector.memset(Ws[at][:].bitcast(u32), 0)
                nc.sync.dma_start(out=Ws[at][:, :1],
                                  in_=Ws_in[at * 128:(at + 1) * 128, :])
                nc.sync.dma_start(out=bqm[at][:],
                                  in_=bqm_in[at * 128:(at + 1) * 128, :])
            Wcall = smallp.tile([128, 8 * C], f32r, tag="Wcall")
            nc.sync.dma_start(out=Wcall[:].rearrange("p (m j) -> p m j", m=8),
                              in_=Wc_in[:].rearrange("(m p) j -> p m j", p=128))
            Wc = [Wcall[:, m * C:(m + 1) * C] for m in range(8)]
            ones = smallp.tile([128, 2], f32r, tag="ones")
            nc.vector.memset(ones[:].bitcast(u32), 0)
            nc.vector.memset(ones[:, :1].bitcast(u32), 0x3F800000)
            # mask4[p, j] = 1.0 iff j == p // 32
            mask4 = smallp.tile([128, 4], f32, tag="mask4")
            nc.vector.memset(mask4[:], 1.0)
            nc.gpsimd.affine_select(out=mask4[:], in_=mask4[:],
                                    compare_op=mybir.AluOpType.is_ge, fill=0.0,
                                    base=0, pattern=[[-32, 4]], channel_multiplier=1)
            nc.gpsimd.affine_select(out=mask4[:], in_=mask4[:],
                                    compare_op=mybir.AluOpType.is_ge, fill=0.0,
                                    base=31, pattern=[[32, 4]], channel_multiplier=-1)
            ident = smallp.tile([128, 128], f32, tag="ident")
            make_identity(nc, ident[:])

            # ---- qprojT [2][128a, 32] ----
            qprojT = [smallp.tile([128, BROWS], f32, tag=f"qp{at}", name=f"qpt{at}") for at in range(2)]
            for at in range(2):
                ps = psump.tile([128, BROWS], f32, tag="ps_a")
                for dc in range(4):
                    nc.tensor.matmul(
                        ps[:],
                        lhsT=Wq[dc][:, at * 128:(at + 1) * 128],
                        rhs=qT[dc][:, :BROWS],
                        start=(dc == 0), stop=(dc == 3))
                nc.scalar.copy(out=qprojT[at][:], in_=ps[:])

            # ---- hT = tanh(kprojT + qprojT_bcast + bqm) ; scores ----
            sc_ps = psump.tile([128, NCD], f32, tag="ps_sc")
            for at in range(2):
                kp = psump.tile([128, NCD], f32, tag="ps_kp", bufs=2)
                for dc in range(4):
                    for half in range(2):
                        nc.tensor.matmul(
                            kp[:, half * 512:(half + 1) * 512],
                            lhsT=Wm[dc][:, at * 128:(at + 1) * 128],
                            rhs=knnT[dc][:, half * 512:(half + 1) * 512],
                            start=(dc == 0), stop=(dc == 3))
                hT = bigp.tile([128, NCD], f32r, tag=f"hT{at}")
                qb = qprojT[at][:, :, None].to_broadcast([128, BROWS, K])
                nc.vector.tensor_tensor(
                    hT[:].rearrange("p (q k) -> p q k", k=K),
                    kp[:].rearrange("p (q k) -> p q k", k=K),
                    qb, mybir.AluOpType.add)
                nc.scalar.activation(hT[:], hT[:], mybir.ActivationFunctionType.Tanh,
                                     bias=bqm[at][:])
                for half in range(2):
                    nc.tensor.matmul(
                        sc_ps[:, half * 512:(half + 1) * 512],
                        lhsT=Ws[at][:],
                        rhs=hT[:, half * 512:(half + 1) * 512],
                        start=(at == 0), stop=(at == 1))
            e_row = smallp.tile([1, NCD], f32, tag="e_row")
            nc.scalar.activation(e_row[:], sc_ps[:1, :],
                                 mybir.ActivationFunctionType.Exp)
            # bounce through DRAM to redistribute [1, 1024] -> [128, 8]
            nc.sync.dma_start(out=escratch[:, :], in_=e_row[:, :])
            e_col = smallp.tile([128, 8], f32, tag="e_col")
            nc.sync.dma_start(out=e_col[:],
                              in_=escratch[0, :].rearrange("(t p) -> p t", p=128))

            # ---- block-diag softmax weights (M-padded), den, attended ----
            w2 = [bigp.tile([128, 128], f32r, tag=f"w2_{t}", name=f"w2t{t}") for t in range(8)]
            for t in range(8):
                nc.vector.memset(w2[t][:].bitcast(u32), 0)
                nc.vector.tensor_scalar_mul(w2[t][:, 4 * t:4 * t + 4], mask4[:],
                                            e_col[:, t:t + 1])
            den_ps = psump.tile([128, 2], f32, tag="ps_a")
            for t in range(8):
                nc.tensor.matmul(den_ps[:], lhsT=w2[t][:], rhs=ones[:],
                                 start=(t == 0), stop=(t == 7))
            att_ps = psump.tile([128, D], f32, tag="ps_kp", bufs=2)
            for t in range(8):
                nc.tensor.matmul(att_ps[:], lhsT=w2[t][:], rhs=knn[t],
                                 start=(t == 0), stop=(t == 7))
            rden = smallp.tile([BROWS, 1], f32, tag="rden")
            nc.vector.reciprocal(rden[:], den_ps[:BROWS, :1])
            att = smallp.tile([BROWS, D], f32, tag="att_sb")
            nc.vector.tensor_scalar_mul(att[:], att_ps[:BROWS, :], rden[:])

            # ---- attendedT via PE transpose (plain fp32) ----
            attT = [smallp.tile([128, 128], f32r, tag=f"attT{dc}", name=f"attTt{dc}") for dc in range(4)]
            for dc in range(4):
                tp = psump.tile([128, BROWS], f32, tag="ps_a")
                nc.tensor.transpose(tp[:], att[:, dc * 128:(dc + 1) * 128],
                                    ident[:BROWS, :BROWS])
                nc.vector.memset(attT[dc][:].bitcast(u32), 0)
                nc.scalar.copy(out=attT[dc][:, :BROWS], in_=tp[:])

            # ---- classifier ----
            out_ps = psump.tile([128, C], f32, tag="ps_out")
            for m in range(8):
                lhsT = qT[m] if m < 4 else attT[m - 4]
                nc.tensor.matmul(out_ps[:], lhsT=lhsT[:], rhs=Wc[m],
                                 start=(m == 0), stop=(m == 7))
            out_sb = smallp.tile([BROWS, C], f32, tag="out_sb")
            nc.scalar.copy(out=out_sb[:], in_=out_ps[:BROWS, :])
            nc.sync.dma_start(out=out_d[:, :], in_=out_sb[:])
    nc.finalize()
    return nc


def _phase1_nc():
    global _PH1
    if _PH1 is None:
        _PH1 = _build_phase1()
    return _PH1


def _phase2_nc():
    global _PH2
    if _PH2 is None:
        _PH2 = _build_phase2()
    return _PH2


def kernel(query_feat, memory_keys, Wq, bq, Wm, bm, Ws, bs, Wc, bc):
    query_feat = np.asarray(query_feat, np.float32)
    memory_keys = np.asarray(memory_keys, np.float32)

    # ---- host prep: pad + normalize + transpose + shard keys ----
    kn = np.sqrt((memory_keys ** 2).sum(axis=1))
    khat = memory_keys * (1.0 / kn)[:, None]
    pad = np.full((NPAD - N, D), -1.0 / np.sqrt(D), np.float32)
    khat_pad = np.concatenate([khat.astype(np.float32), pad], axis=0)
    qT_full = np.ascontiguousarray(query_feat.T)  # [512, 256]

    ph1 = _phase1_nc()
    in_maps = []
    for c in range(NC_CORES):
        sh = khat_pad[c * SHARD:(c + 1) * SHARD]          # [12800, 512]
        arr = np.ascontiguousarray(
            sh.reshape(NCHUNK, CHUNK, 4, 128).transpose(0, 3, 2, 1)
        ).reshape(NCHUNK, 128, 4 * CHUNK)
        in_maps.append({"khatT": arr, "qT": qT_full})
    res1 = run_bass_kernel_spmd(ph1, in_maps, core_ids=list(range(NC_CORES)))

    # ---- host merge: recover indices, exact re-score of candidates ----
    all_gidx = np.zeros((B, NC_CORES, KLOC), np.int64)
    for c in range(NC_CORES):
        win = res1.results[c]["win"].view(np.uint32)
        pos = res1.results[c]["pos"].astype(np.int64)   # 0..399 in L1
        seg = pos // 8
        within = (win & np.uint32(0x1FF)).astype(np.int64)
        all_gidx[:, c, :] = seg * SEG + within + c * SHARD
    gidx = all_gidx.reshape(B, CAND)
    safe = np.minimum(gidx, N - 1)
    q32 = np.maximum(query_feat, 0)
    cand_keys = memory_keys[safe]                       # [256, 320, 512]
    dots = np.einsum("bd,bcd->bc", q32, cand_keys, optimize=True)
    cos = dots / np.maximum(
        np.linalg.norm(q32, axis=1)[:, None] * kn[safe], np.float32(1e-8))
    cos[gidx >= N] = -np.inf                            # mask dummy-pad hits
    order = np.argsort(-cos, axis=1, kind="stable")[:, :K]
    top_idx = np.take_along_axis(safe, order, axis=1)   # [256, 32]
    knn = memory_keys[top_idx]                          # [256, 32, 512]

    # ---- phase 2 (batch sharded) ----
    ph2 = _phase2_nc()
    bqm = (np.asarray(bq, np.float32) + np.asarray(bm, np.float32)).reshape(A, 1)
    Wq_a = np.ascontiguousarray(np.asarray(Wq, np.float32))
    Wm_a = np.ascontiguousarray(np.asarray(Wm, np.float32))
    Ws_a = np.ascontiguousarray(np.asarray(Ws, np.float32))
    Wc_a = np.ascontiguousarray(np.asarray(Wc, np.float32))
    in_maps2 = []
    for c in range(NC_CORES):
        rows = slice(c * BROWS, (c + 1) * BROWS)
        knn_c = knn[rows].reshape(BROWS * K, D)
        in_maps2.append({
            "qT": np.ascontiguousarray(query_feat[rows].T),
            "knn": np.ascontiguousarray(knn_c),
            "knnT": np.ascontiguousarray(knn_c.T),
            "Wq": Wq_a, "Wm": Wm_a, "Ws": Ws_a, "bqm": bqm, "Wc": Wc_a,
        })
    res2 = run_bass_kernel_spmd(ph2, in_maps2, core_ids=list(range(NC_CORES)))
    out = np.concatenate([res2.results[c]["out"] for c in range(NC_CORES)], axis=0)
    return (out + np.asarray(bc, np.float32)[None, :]).astype(np.float32)



# revision 5
# speedup vs baseline: 2.1845x; 2.1845x over previous
"""Trainium2 Bass kernel for retrieval-knn attention classifier (nn_MA_51866025067137).

Strategy (8 NeuronCores):
  Phase 1 — memory_keys sharded along N (12800 keys/core, padded 100000->102400
  with zero rows).  Keys are L2-normalized and quantized to fp8e4m3 on host;
  each core computes ranking sims for all 256 queries against its shard with
  fp8 DoubleRow matmuls (2 rows/partition, 0.5 cyc/col) and the ACT engine
  evicts each PSUM chunk as Relu(sim - tau_b) in fp8 — a per-key candidate
  flag.  tau_b is a per-query statistical threshold (mu + z*sigma, estimated
  host-side from a key sample) tuned to flag ~50 keys/core/query.  The flag
  map (1 B/key) is DMA'd out; the host scans nonzeros, re-scores the ~400
  candidates per query exactly in fp32, and takes the global top-32.
  Phase 2 — batch sharded (32 queries/core): memory-attention module + classifier
  entirely in bf16 matmuls from a single weight blob: scores are computed in
  transposed layout (no DRAM bounce), attended^T is computed directly via
  per-tile matmuls against the softmax block weights (no PE transposes), and
  the 1/den normalization is folded into a final fused multiply-add.
"""

import numpy as np
import ml_dtypes

import concourse.bacc as bacc
import concourse.mybir as mybir
from concourse.tile import TileContext
from concourse.bass_utils import run_bass_kernel_spmd

# problem dims (hardcoded per harness contract)
B, N, D = 256, 100000, 512
A, C, K = 256, 100, 32
NC_CORES = 8
NPAD = 102400             # 8 * 12800
SHARD = NPAD // NC_CORES  # 12800
CHUNK = 512               # keys per matmul chunk
NCHUNK = SHARD // CHUNK   # 25
BROWS = B // NC_CORES     # 32 rows per core in phase 2
NCD = BROWS * K           # 1024
ZTHRESH = 2.62            # flag-rate z-score (~4.4e-3 -> ~56 flags/core/row)
NSAMP = 2048              # host-side sample size for per-row sim stats

f32 = mybir.dt.float32
bf16 = mybir.dt.bfloat16
f8 = mybir.dt.float8e4
u32 = mybir.dt.uint32
F8NP = ml_dtypes.float8_e4m3
BF16NP = ml_dtypes.bfloat16

_PH1 = None
_PH2 = None

# weight-blob column offsets (phase 2)
WM_OFF = 0                # 4 x [128, 256]
WS_OFF = 1024             # 2 x [128, 1]
WQ_OFF = 1026             # 4 x [128, 256]
QT_OFF = 2050             # 4 x [128, 32]
WBA_COLS = 2178           # first dma piece
WC_OFF = 2178             # 8 x [128, 100]
WB_COLS = 2978


def _build_phase1():
    nc = bacc.Bacc("TRN2", target_bir_lowering=False)
    k8_d = nc.dram_tensor("k8", [NCHUNK, 128, 4 * CHUNK], f8, kind="ExternalInput")
    q8_d = nc.dram_tensor("q8", [128, 1024], f8, kind="ExternalInput")
    tau_d = nc.dram_tensor("tau", [128, 2], f32, kind="ExternalInput")
    fl_d = nc.dram_tensor("fl", [2, 128, SHARD], f8, kind="ExternalOutput")

    # dump flag columns in 4 pieces per qt so the out-DMAs overlap compute
    pieces = [(0, 7), (7, 13), (13, 19), (19, 25)]

    with TileContext(nc) as tc:
        with (
            tc.tile_pool(name="qp", bufs=1) as qp,
            tc.tile_pool(name="keys", bufs=4) as keyp,
            tc.tile_pool(name="fl", bufs=1) as flp,
            tc.tile_pool(name="psum", bufs=4, space="PSUM") as psump,
        ):
            q8 = qp.tile([128, 1024], f8, tag="q8")
            nc.sync.dma_start(out=q8[:], in_=q8_d[:, :])
            tau = qp.tile([128, 2], f32, tag="tau")
            nc.sync.dma_start(out=tau[:], in_=tau_d[:, :])
            q8v = q8[:].rearrange("p (mc two b) -> p mc two b", mc=2, two=2)

            fl = [flp.tile([128, SHARD], f8, tag=f"fl{qt}", name=f"flt{qt}")
                  for qt in range(2)]

            for c in range(NCHUNK):
                kt = keyp.tile([128, 4 * CHUNK], f8, tag="kt")
                nc.sync.dma_start(out=kt[:], in_=k8_d[c, :, :])
                ktv = kt[:].rearrange("p (mc two j) -> p mc two j", mc=2, two=2)
                for qt in range(2):
                    ps = psump.tile([128, CHUNK], f32, tag=f"ps{qt}")
                    for mc in range(2):
                        nc.tensor.matmul(
                            ps[:],
                            lhsT=q8v[:, mc, :, qt * 128:(qt + 1) * 128],
                            rhs=ktv[:, mc, :, :],
                            start=(mc == 0), stop=(mc == 1),
                            perf_mode=mybir.MatmulPerfMode.DoubleRow,
                        )
                    nc.scalar.activation(
                        fl[qt][:, c * CHUNK:(c + 1) * CHUNK], ps[:],
                        mybir.ActivationFunctionType.Relu,
                        bias=tau[:, qt:qt + 1])
                for (c0, c1) in pieces:
                    if c1 == c + 1:
                        for qt in range(2):
                            nc.sync.dma_start(
                                out=fl_d[qt, :, c0 * CHUNK:c1 * CHUNK],
                                in_=fl[qt][:, c0 * CHUNK:c1 * CHUNK])
    nc.finalize()
    return nc


def _build_phase2():
    nc = bacc.Bacc("TRN2", target_bir_lowering=False)
    wb_d = nc.dram_tensor("wb", [128, WB_COLS], bf16, kind="ExternalInput")
    bqm_d = nc.dram_tensor("bqm", [128, 2], f32, kind="ExternalInput")
    knnT_d = nc.dram_tensor("knnT", [128, 4 * NCD], bf16, kind="ExternalInput")
    knn_d = nc.dram_tensor("knn", [128, 8 * D], bf16, kind="ExternalInput")
    out_d = nc.dram_tensor("out", [BROWS, C], f32, kind="ExternalOutput")

    with TileContext(nc) as tc:
        with (
            tc.tile_pool(name="sb", bufs=1) as sb,
            tc.tile_pool(name="kp_ps", bufs=1, space="PSUM") as kpp,
            tc.tile_pool(name="ps1", bufs=1, space="PSUM") as ps1,
        ):
            # ---- constants (off critical path) ----
            # Bmat [32, 1024]: B[b, cand] = 1 iff cand // 32 == b
            Bmat = sb.tile([32, NCD], bf16, tag="Bmat")
            nc.vector.memset(Bmat[:], 1.0)
            Bv = Bmat[:].rearrange("p (j1 j2) -> p j1 j2", j1=32)
            nc.gpsimd.affine_select(out=Bv, in_=Bv,
                                    compare_op=mybir.AluOpType.is_ge, fill=0.0,
                                    base=0, pattern=[[1, 32], [0, 32]],
                                    channel_multiplier=-1)
            nc.gpsimd.affine_select(out=Bv, in_=Bv,
                                    compare_op=mybir.AluOpType.is_ge, fill=0.0,
                                    base=0, pattern=[[-1, 32], [0, 32]],
                                    channel_multiplier=1)
            # maskwide [128, 60]: M[p, c] = 1 iff c == p//32 + 28
            mw = sb.tile([128, 60], bf16, tag="mw")
            nc.vector.memset(mw[:], 1.0)
            nc.gpsimd.affine_select(out=mw[:], in_=mw[:],
                                    compare_op=mybir.AluOpType.is_ge, fill=0.0,
                                    base=896, pattern=[[-32, 60]],
                                    channel_multiplier=1)
            nc.gpsimd.affine_select(out=mw[:], in_=mw[:],
                                    compare_op=mybir.AluOpType.is_ge, fill=0.0,
                                    base=-865, pattern=[[32, 60]],
                                    channel_multiplier=-1)
            ones1 = sb.tile([128, 1], bf16, tag="ones1")
            nc.vector.memset(ones1[:], 1.0)

            # ---- inputs ----
            wb = sb.tile([128, WB_COLS], bf16, tag="wb")
            bqm = sb.tile([128, 2], f32, tag="bqm")
            knnT = sb.tile([128, 4 * NCD], bf16, tag="knnT")
            knn = sb.tile([128, 8 * D], bf16, tag="knn")
            nc.sync.dma_start(out=bqm[:], in_=bqm_d[:, :])
            nc.sync.dma_start(out=wb[:, :WBA_COLS], in_=wb_d[:, :WBA_COLS])
            nc.sync.dma_start(out=knnT[:], in_=knnT_d[:, :])
            nc.sync.dma_start(out=wb[:, WBA_COLS:], in_=wb_d[:, WBA_COLS:])
            nc.sync.dma_start(out=knn[:], in_=knn_d[:, :])

            def wm(dc, at):
                off = WM_OFF + dc * 256 + at * 128
                return wb[:, off:off + 128]

            def wq(dc):
                off = WQ_OFF + dc * 256
                return wb[:, off:off + 256]

            def qt_(dc):
                off = QT_OFF + dc * 32
                return wb[:, off:off + 32]

            def wc(m):
                off = WC_OFF + m * 100
                return wb[:, off:off + 100]

            # ---- qprojT-by-rows: qpb [32, 256] = q @ Wq ----
            qpb_ps = ps1.tile([32, A], f32, tag="qpb")
            for dc in range(4):
                nc.tensor.matmul(qpb_ps[:], lhsT=qt_(dc), rhs=wq(dc),
                                 start=(dc == 0), stop=(dc == 3))
            qpb = sb.tile([32, A], bf16, tag="qpb_sb")
            nc.scalar.copy(out=qpb[:], in_=qpb_ps[:])

            # ---- outq = q @ Wc[:D] (early; needs only wb) ----
            outq_ps = ps1.tile([BROWS, C], f32, tag="outq")
            for dc in range(4):
                nc.tensor.matmul(outq_ps[:], lhsT=qt_(dc), rhs=wc(dc),
                                 start=(dc == 0), stop=(dc == 3))
            outq = sb.tile([BROWS, C], f32, tag="outq_sb")
            nc.scalar.copy(out=outq[:], in_=outq_ps[:])

            # ---- kp[at] = (knn @ Wm + qproj broadcast); hT = tanh(kp + bqm) ----
            hT = [sb.tile([128, NCD], bf16, tag=f"hT{at}", name=f"hTt{at}")
                  for at in range(2)]
            for at in range(2):
                kp = kpp.tile([128, NCD], f32, tag="kp")
                for half in range(2):
                    sl = slice(half * 512, (half + 1) * 512)
                    for dc in range(4):
                        nc.tensor.matmul(
                            kp[:, sl], lhsT=wm(dc, at), rhs=knnT[:, dc * NCD:][:, sl],
                            start=(dc == 0), stop=False)
                    nc.tensor.matmul(
                        kp[:, sl], lhsT=qpb[:, at * 128:(at + 1) * 128],
                        rhs=Bmat[:, sl], start=False, stop=True,
                        skip_group_check=True)
                nc.scalar.activation(hT[at][:], kp[:],
                                     mybir.ActivationFunctionType.Tanh,
                                     bias=bqm[:, at:at + 1])

            # ---- scoresT [128, 8] -> e = exp ----
            sc_ps = ps1.tile([128, 8], f32, tag="sc")
            for t in range(8):
                for at in range(2):
                    nc.tensor.matmul(
                        sc_ps[:, t:t + 1],
                        lhsT=hT[at][:, t * 128:(t + 1) * 128],
                        rhs=wb[:, WS_OFF + at:WS_OFF + at + 1],
                        start=(at == 0), stop=(at == 1))
            e_sb = sb.tile([128, 8], f32, tag="e_sb")
            nc.scalar.activation(e_sb[:], sc_ps[:],
                                 mybir.ActivationFunctionType.Exp)

            # ---- block softmax weights w2 [128, 256]: w2[p, t*32+b] ----
            w2 = sb.tile([128, 256], bf16, tag="w2")
            for t in range(8):
                nc.vector.tensor_scalar_mul(
                    w2[:, t * 32:(t + 1) * 32],
                    mw[:, 28 - 4 * t:60 - 4 * t],
                    e_sb[:, t:t + 1])

            # ---- den, attT (unscaled), classifier ----
            den_ps = ps1.tile([BROWS, 1], f32, tag="den")
            for t in range(8):
                nc.tensor.matmul(den_ps[:], lhsT=w2[:, t * 32:(t + 1) * 32],
                                 rhs=ones1[:], start=(t == 0), stop=(t == 7))
            rden = sb.tile([BROWS, 1], f32, tag="rden")
            nc.vector.reciprocal(rden[:], den_ps[:])

            attT_ps = ps1.tile([128, 128], f32, tag="attT")
            for dc in range(4):
                for t in range(8):
                    nc.tensor.matmul(
                        attT_ps[:, dc * 32:(dc + 1) * 32],
                        lhsT=knn[:, t * D + dc * 128:t * D + (dc + 1) * 128],
                        rhs=w2[:, t * 32:(t + 1) * 32],
                        start=(t == 0), stop=(t == 7))
            attT = sb.tile([128, 128], bf16, tag="attT_sb")
            nc.scalar.copy(out=attT[:], in_=attT_ps[:])

            outa_ps = ps1.tile([BROWS, C], f32, tag="outa")
            for dc in range(4):
                nc.tensor.matmul(outa_ps[:], lhsT=attT[:, dc * 32:(dc + 1) * 32],
                                 rhs=wc(4 + dc), start=(dc == 0), stop=(dc == 3))

            # out = outa * rden + outq  (fold 1/den at the end)
            out_sb = sb.tile([BROWS, C], f32, tag="out_sb")
            nc.vector.scalar_tensor_tensor(
                out=out_sb[:], in0=outa_ps[:], scalar=rden[:], in1=outq[:],
                op0=mybir.AluOpType.mult, op1=mybir.AluOpType.add)
            nc.sync.dma_start(out=out_d[:, :], in_=out_sb[:])
    nc.finalize()
    return nc


def _phase1_nc():
    global _PH1
    if _PH1 is None:
        _PH1 = _build_phase1()
    return _PH1


def _phase2_nc():
    global _PH2
    if _PH2 is None:
        _PH2 = _build_phase2()
    return _PH2


def kernel(query_feat, memory_keys, Wq, bq, Wm, bm, Ws, bs, Wc, bc):
    query_feat = np.asarray(query_feat, np.float32)
    memory_keys = np.asarray(memory_keys, np.float32)

    # ---- host prep: normalize keys, quantize to fp8, DoubleRow layout ----
    kn = np.sqrt((memory_keys ** 2).sum(axis=1))
    khat = memory_keys * (1.0 / kn)[:, None]
    khat_pad = np.zeros((NPAD, D), np.float32)
    khat_pad[:N] = khat
    k8 = khat_pad.astype(F8NP)

    q32 = np.maximum(query_feat, 0)
    q8 = q32.astype(F8NP)

    # per-row flag threshold tau_b = mu_b + z * sigma_b of the fp8 sims,
    # estimated from a uniform key sample (exact same arrays the PE sees)
    k8f = k8[:N].astype(np.float32)
    q8f = q8.astype(np.float32)
    samp = k8f[:: N // NSAMP][:NSAMP]
    sims_s = q8f @ samp.T                              # [B, NSAMP]
    mu = sims_s.mean(axis=1)
    sig = sims_s.std(axis=1)
    tau = (mu + ZTHRESH * sig).astype(np.float32)      # [B]
    # bias for ACT: Relu(sim + bias), bias = -tau, laid out [128, 2(qt)]
    bias = (-tau).reshape(2, 128).T.copy()             # [128, 2]

    # q8 DoubleRow layout: [128p, (mc two b)]
    q8arr = np.ascontiguousarray(
        q8.T.reshape(2, 2, 128, B).transpose(2, 0, 1, 3)).reshape(128, 1024)

    ph1 = _phase1_nc()
    in_maps = []
    for c in range(NC_CORES):
        sh = k8[c * SHARD:(c + 1) * SHARD]             # [12800, 512]
        arr = np.ascontiguousarray(
            sh.reshape(NCHUNK, CHUNK, 2, 2, 128).transpose(0, 4, 2, 3, 1)
        ).reshape(NCHUNK, 128, 4 * CHUNK)
        in_maps.append({"k8": arr, "q8": q8arr, "tau": bias})
    res1 = run_bass_kernel_spmd(ph1, in_maps, core_ids=list(range(NC_CORES)))

    # ---- host: scan flags, exact re-score, global top-32 ----
    flags = np.empty((B, NPAD), np.uint8)
    for c in range(NC_CORES):
        fl = res1.results[c]["fl"].view(np.uint8)      # [2, 128, SHARD]
        flags[:, c * SHARD:(c + 1) * SHARD] = fl.reshape(B, SHARD)
    flags &= 0x7F                                      # ignore sign bit of -0
    rows, cols = np.nonzero(flags)
    cnt = np.bincount(rows, minlength=B)
    Mx = max(int(cnt.max()), K)
    idxpad = np.zeros((B, Mx), np.int64)
    mask = np.arange(Mx)[None, :] < cnt[:, None]
    idxpad[mask] = cols
    ok = mask & (idxpad < N)

    qn = np.sqrt((q32 ** 2).sum(axis=1))
    safe = np.minimum(idxpad, N - 1)
    cand_keys = memory_keys[safe]                      # [B, Mx, D]
    dots = np.einsum("bd,bmd->bm", q32, cand_keys, optimize=True)
    cos = dots / np.maximum(qn[:, None] * kn[safe], np.float32(1e-8))
    cos[~ok] = -np.inf

    short = np.nonzero(ok.sum(axis=1) < K)[0]
    if short.size:                                     # statistical fallback
        sims_full = q32[short] @ memory_keys.T
        cos_full = sims_full / np.maximum(
            qn[short, None] * kn[None, :], np.float32(1e-8))
        top_f = np.argsort(-cos_full, axis=1, kind="stable")[:, :K]
    order = np.argsort(-cos, axis=1, kind="stable")[:, :K]
    top_idx = np.take_along_axis(safe, order, axis=1)  # [B, K]
    if short.size:
        top_idx[short] = top_f
    knn = memory_keys[top_idx]                         # [B, K, D]

    # ---- phase 2 (batch sharded, bf16 blob) ----
    ph2 = _phase2_nc()
    Wq_a = np.asarray(Wq, np.float32)
    Wm_a = np.asarray(Wm, np.float32)
    Ws_a = np.asarray(Ws, np.float32).reshape(A)
    Wc_a = np.asarray(Wc, np.float32)
    bqm = (np.asarray(bq, np.float32) + np.asarray(bm, np.float32))
    bqm_arr = np.ascontiguousarray(bqm.reshape(2, 128).T)          # [128, 2]

    wb = np.zeros((128, WB_COLS), np.float32)
    wb[:, WM_OFF:WM_OFF + 1024] = Wm_a.reshape(4, 128, A).transpose(1, 0, 2).reshape(128, 1024)
    wb[:, WS_OFF:WS_OFF + 2] = Ws_a.reshape(2, 128).T
    wb[:, WQ_OFF:WQ_OFF + 1024] = Wq_a.reshape(4, 128, A).transpose(1, 0, 2).reshape(128, 1024)
    wb[:, WC_OFF:WC_OFF + 800] = Wc_a.reshape(8, 128, C).transpose(1, 0, 2).reshape(128, 800)

    in_maps2 = []
    for c in range(NC_CORES):
        rows2 = slice(c * BROWS, (c + 1) * BROWS)
        wb_c = wb.copy()
        wb_c[:, QT_OFF:QT_OFF + 128] = (
            q32[rows2].T.reshape(4, 128, BROWS).transpose(1, 0, 2).reshape(128, 128))
        knn_c = knn[rows2].reshape(NCD, D)             # [1024, 512]
        knnT_c = np.ascontiguousarray(
            knn_c.T.reshape(4, 128, NCD).transpose(1, 0, 2).reshape(128, 4 * NCD))
        knn_l = np.ascontiguousarray(
            knn_c.reshape(8, 128, D).transpose(1, 0, 2).reshape(128, 8 * D))
        in_maps2.append({
            "wb": wb_c.astype(BF16NP),
            "bqm": bqm_arr,
            "knnT": knnT_c.astype(BF16NP),
            "knn": knn_l.astype(BF16NP),
        })
    res2 = run_bass_kernel_spmd(ph2, in_maps2, core_ids=list(range(NC_CORES)))
    out = np.concatenate([res2.results[c]["out"] for c in range(NC_CORES)], axis=0)
    return (out + np.asarray(bc, np.float32)[None, :]).astype(np.float32)
